# revision 2
# baseline (speedup 1.0000x reference)
import numpy as np

import concourse.mybir as mybir
import concourse.tile as tile
from concourse import bacc
from concourse.bass_utils import run_bass_kernel_spmd
from concourse.kernels.tile_matmul import matmul_tile_kernel

# y = sum_w x[w] @ weight[w].T + sum_w bias[w], reshaped to [W, M/W, N].
# Fold the rank sum into the contraction: K_tot = W*K, one GEMM per core.
# Shard M=4096 into 8 chunks of 512 — no cross-core communication needed.
W, M, K, N = 4, 4096, 2048, 4096
NCORES = 8
MC = M // NCORES        # 512 output rows per core
KT = W * K              # 8192 contraction dim
P = 128

_compiled = None


def _build():
    nc = bacc.Bacc(None, target_bir_lowering=False)
    with tile.TileContext(nc) as tc:
        with tc.tile_pool(name="dram", bufs=1, space="DRAM") as dram:
            kxm = dram.tile((P, KT // P, MC), mybir.dt.float32, kind="ExternalInput")
            kxn = dram.tile((P, KT // P, N), mybir.dt.float32, kind="ExternalInput")
            bacc_in = dram.tile((P, MC // P, N), mybir.dt.float32, kind="ExternalInput")
            mxn = dram.tile((P, MC // P, N), mybir.dt.float32, kind="ExternalOutput")
            matmul_tile_kernel(tc, kxm[:], kxn[:], mxn[:], accumulate_ap=bacc_in[:],
                               cache_tiles=False)
    nc.compile()
    return nc, kxm.name, kxn.name, bacc_in.name, mxn.name


def _get_compiled():
    global _compiled
    if _compiled is None:
        _compiled = _build()
    return _compiled


def _kmajor(a, cols):
    # logical [KT, cols] -> stored [P, KT//P, cols] with k = ko*P + p
    return np.ascontiguousarray(a.reshape(KT // P, P, cols).transpose(1, 0, 2))


def kernel(x, weight, bias):
    nc, kxm_name, kxn_name, bacc_name, mxn_name = _get_compiled()

    xt = x.transpose(0, 2, 1).reshape(KT, M)           # [KT, M], k-major over (w,k)
    wt = weight.transpose(0, 2, 1).reshape(KT, N)      # [KT, N]
    bsum = bias.sum(axis=0, dtype=np.float32)          # [M, N]

    kxn_np = _kmajor(wt, N)                            # shared by all cores
    in_maps = []
    for c in range(NCORES):
        m0 = c * MC
        kxm_np = _kmajor(np.ascontiguousarray(xt[:, m0:m0 + MC]), MC)
        b = bsum[m0:m0 + MC]
        b_np = np.ascontiguousarray(b.reshape(MC // P, P, N).transpose(1, 0, 2))
        in_maps.append({kxm_name: kxm_np, kxn_name: kxn_np, bacc_name: b_np})

    res = run_bass_kernel_spmd(nc, in_maps, core_ids=list(range(NCORES)))

    chunks = []
    for c in range(NCORES):
        o = res.results[c][mxn_name]                   # [P, MC//P, N]
        chunks.append(o.transpose(1, 0, 2).reshape(MC, N))
    y = np.concatenate(chunks, axis=0)                 # [M, N]
    return y.reshape(W, M // W, N).astype(np.float32)
